# revision 5
# baseline (speedup 1.0000x reference)
"""Multi-head causal attention (B=4, T=2048, C=1024, H=16, D=64) on 8 TRN2 cores.

Sharding: core c = (batch b = c//2, head-group g = c%2 of 8 heads).
Per core (its batch, its 8 heads), all matmuls in bf16 with fp32 PSUM accum.

Structure (v4): two PE tiling modes only, to minimise mode-switch drains:
  - scores: 64x128 row-tiled pairs (head 2p in T0 / head 2p+1 in T8), exactly
    as v3 -- the two heads' QK^T matmuls run concurrently.
  - EVERYTHING else (QKV fills, AV, denominators, output proj, warmup) runs
    128x32 column-tiled: 4 concurrent matmuls per slot, each writing a
    32-partition strip of one PSUM bank.  Legal because concurrent tiles may
    share a bank only when they write disjoint partition strips.
Per 2-step batch the PE switches row<->col mode exactly twice.

AV packing: one slot per s-chunk computes BOTH heads' AV (h0 -> psum parts
0:64, h1 -> 64:128 of po_num), halving AV time vs v3.  Denominators no longer
ride as ones-columns; instead one 128x32 slot per batch accumulates
  strip q = denom(head q&1, s-chunks of slot q>>1)     (ones32 lhsT, M=32)
into a separate bank, and at block end a tiny mask matmul (mask32, entries
1/32) sums the even/odd strips and replicates each head's denominator across
its 64 num partitions.  The normalize then needs no partition-shift copies:
  rcp = recip(den2), OT[pr] = po_num * rcp   (single [128,SW] ops).

  QT/KT = W^T x^T            [E=512, T] head-major rows (bf16)
  V     = x Wv               [T, E] -> Vsb[s, sc, h, d]
  attention in (t-slab j of 512) x (pair p) blocks, slab-major; per batch m:
    scores(2m), scores(2m+1) row-tiled; lagged AV+den slots and paced fillers
    col-tiled.  Diag masks via gpsimd affine_select on pt (hidden by the lag).
  y = OT^T Wo (per 128-row chunk), written bf16; host sums the two
  head-group partials + bias in f32.

OT is split per head-pair (OTp[0..2] full, OTp3 slabs 0-2 + OT3c per-128
chunks for slab 3) so the final projection tail never waits on unrelated
OT writes; the last block's normalize runs per 128-chunk, pipelined with the
ko=3 tail matmuls, keeping the PE warm through the tail.

Inputs are repacked host-side into partition-major layouts so every DMA is
contiguous per partition.  QKV slab fills and the output projection are paced
into the attention stream as filler pieces with batch-granular deadlines.
"""

from collections import deque

import ml_dtypes
import numpy as np

import concourse.bacc as bacc
import concourse.mybir as mybir
import concourse.tile as tile
from concourse.bass_utils import run_bass_kernel_spmd

B, T, C, H, D = 4, 2048, 1024, 16, 64
NH = 8                 # heads per core
E = NH * D             # 512 per-core head width
P = 128
KO = C // P            # 8 contraction chunks for QKV proj
ET = E // P            # 4 e-tiles == head pairs
SW = 512               # psum-bank width / t-slab width
NSLAB = T // SW        # 4
NSC = T // P           # 16 s-chunks
LOOKAHEAD = 2          # AV emission lag (batches)
F32 = mybir.dt.float32
BF16 = mybir.dt.bfloat16
EXP = mybir.ActivationFunctionType.Exp
GE = mybir.AluOpType.is_ge
SCALE = float(D) ** -0.5
BF16NP = ml_dtypes.bfloat16

_CACHE: dict = {}


def _build():
    nc = bacc.Bacc("TRN2", target_bir_lowering=False, debug=False)
    # partition-major packed layouts (host rearranges; all DMA slices are
    # per-partition contiguous)
    xt_d = nc.dram_tensor("xt", [P, NSLAB, KO, SW], BF16, kind="ExternalInput")
    wq_d = nc.dram_tensor("wq", [P, ET, KO, P], BF16, kind="ExternalInput")
    wk_d = nc.dram_tensor("wk", [P, ET, KO, P], BF16, kind="ExternalInput")
    wv_d = nc.dram_tensor("wv", [P, KO, E], BF16, kind="ExternalInput")
    wo_d = nc.dram_tensor("wo", [P, ET, C], BF16, kind="ExternalInput")
    y_d = nc.dram_tensor("y", [T, C], BF16, kind="ExternalOutput")

    xt_v = xt_d.ap()
    wq_v = wq_d.ap()
    wk_v = wk_d.ap()
    wv_v = wv_d.ap()
    wo_v = wo_d.ap()
    y_v = y_d.ap()

    def cmm(out, lhsT, rhs, start, stop, nstrip=4):
        # col-tiled slot: nstrip concurrent matmuls, strip q covering output
        # partitions [32q, 32q+32) via lhsT columns [32q, 32q+32)
        for q in range(nstrip):
            nc.tensor.matmul(
                out[32 * q:32 * (q + 1)], lhsT=lhsT[:, 32 * q:32 * (q + 1)],
                rhs=rhs, start=start, stop=stop, tile_position=(0, 32 * q))

    with tile.TileContext(nc) as tc:
        with (
            tc.tile_pool(name="qkv", bufs=1) as qkv_pool,
            tc.tile_pool(name="vsb", bufs=1) as v_pool,
            tc.tile_pool(name="otp", bufs=1) as ot_pool,
            tc.tile_pool(name="wgt", bufs=1) as w_pool,
            tc.tile_pool(name="xsl", bufs=2) as x_pool,
            tc.tile_pool(name="wop", bufs=1) as wo_pool,
            tc.tile_pool(name="ptl", bufs=6) as pt_pool,
            tc.tile_pool(name="rcs", bufs=4) as r_pool,
            tc.tile_pool(name="ysb", bufs=4) as y_pool,
            tc.tile_pool(name="psw", bufs=2, space="PSUM") as ps_w,
            tc.tile_pool(name="pso", bufs=1, space="PSUM") as ps_o,
            tc.tile_pool(name="psu", bufs=2, space="PSUM") as ps_u,
        ):
            QT = qkv_pool.tile([P, ET, T], BF16)
            KT = qkv_pool.tile([P, ET, T], BF16)
            Vsb = v_pool.tile([P, NSC, NH, D], BF16)
            OTp = [ot_pool.tile([P, T], BF16, name=f"otp{e}")
                   for e in range(ET - 1)]
            OTp3 = ot_pool.tile([P, 3 * SW], BF16, name="otp3")
            OT3c = [ot_pool.tile([P, P], BF16, name=f"ot3c{c}")
                    for c in range(4)]

            def ot_w(pr, t0, cols):
                # normalize-target view of head-pair pr, t-slab cols
                if pr < ET - 1:
                    return OTp[pr][:, t0:t0 + cols]
                return OTp3[:, t0:t0 + cols]

            def ot_r(ko, ttt):
                # proj lhsT view: 128-col chunk ttt of head-pair ko
                if ko < ET - 1:
                    return OTp[ko][:, ttt * P:(ttt + 1) * P]
                if ttt < 12:
                    return OTp3[:, ttt * P:(ttt + 1) * P]
                return OT3c[ttt - 12][:, :]

            ones32 = v_pool.tile([P, 32], BF16)
            nc.gpsimd.memset(ones32[:], 1.0)
            # mask32: den2 = mask32.T @ den_strips; strip q holds
            # denom(head q&1, slot q>>1) replicated 32x, so 1/32 entries sum
            # even+odd slots and re-replicate per head (h0 -> rows 0:64,
            # h1 -> rows 64:128)
            mask32 = v_pool.tile([P, P], BF16)
            nc.gpsimd.memset(mask32[:], 0.0)
            nc.gpsimd.memset(mask32[0:32, 0:64], 1.0 / 32)
            nc.gpsimd.memset(mask32[64:96, 0:64], 1.0 / 32)
            nc.gpsimd.memset(mask32[32:64, 64:128], 1.0 / 32)
            nc.gpsimd.memset(mask32[96:128, 64:128], 1.0 / 32)
            # preload the exp table set during the DMA prologue
            scr = v_pool.tile([P, 1], F32)
            nc.scalar.activation(scr[0:1, 0:1], ones32[0:1, 0:1], EXP)
            # warm the PE HAM clock-gate (cold = 1.2GHz) with dummy matmuls
            # during the otherwise PE-idle DMA prologue (col-tiled so the PE
            # mode matches the first real fills)
            dm = v_pool.tile([P, P], BF16)
            nc.gpsimd.memset(dm[:], 0.0)
            pwarm = ps_u.tile([P, SW], F32, tag="ps")
            for _ in range(34):
                cmm(pwarm[:, 0:P], dm, dm[:, 0:P], True, True)

            wq_s = w_pool.tile([P, ET, KO, P], BF16)
            wk_s = w_pool.tile([P, ET, KO, P], BF16)
            wv_s = w_pool.tile([P, KO, E], BF16)
            wo_s = wo_pool.tile([P, ET, C], BF16)

            xs_map = {}

            def x_dma(sl, lo=0, hi=KO):
                def piece():
                    if sl not in xs_map:
                        xs_map[sl] = x_pool.tile(
                            [P, KO, SW], BF16, tag="xs", name=f"xs{sl}")
                    nc.sync.dma_start(
                        xs_map[sl][:, lo:hi, :], xt_v[:, sl, lo:hi, :])
                return piece

            def qk_fill(sl, et, w_s, dst):
                def piece():
                    xs = xs_map[sl]
                    pq = ps_u.tile([P, SW], F32, tag="ps")
                    for ko in range(KO):
                        cmm(pq[:], w_s[:, et, ko, :], xs[:, ko, :],
                            ko == 0, ko == KO - 1)
                    nc.vector.tensor_copy(
                        dst[:, et, sl * SW:(sl + 1) * SW], pq[:])
                return piece

            def v_fill(sl, si):
                def piece():
                    xs = xs_map[sl]
                    pv = ps_u.tile([P, E], F32, tag="ps")
                    for ko in range(KO):
                        cmm(pv[:], xs[:, ko, si * P:(si + 1) * P],
                            wv_s[:, ko, :], ko == 0, ko == KO - 1)
                    st = sl * (SW // P) + si
                    nc.vector.tensor_copy(
                        Vsb[:, st, :, :],
                        pv[:].rearrange("p (h d) -> p h d", d=D))
                return piece

            def wo_dma(et):
                def piece():
                    nc.sync.dma_start(wo_s[:, et, :], wo_v[:, et, :])
                return piece

            ys_map = {}

            def proj_mms(ttt, jn, py, ko_lo, ko_hi):
                for ko in range(ko_lo, ko_hi):
                    for q in range(4):
                        nc.tensor.matmul(
                            py[32 * q:32 * (q + 1)],
                            lhsT=ot_r(ko, ttt)[:, 32 * q:32 * (q + 1)],
                            rhs=wo_s[:, ko, jn * SW:(jn + 1) * SW],
                            start=(ko == 0), stop=(ko == ET - 1),
                            tile_position=(0, 32 * q))

            def proj_store(ttt, jn, py):
                tb = ttt // 2
                if tb not in ys_map:
                    ys_map[tb] = y_pool.tile(
                        [P, 2, C], BF16, tag="ys", name=f"ys{tb}")
                ys = ys_map[tb]
                nc.vector.tensor_copy(
                    ys[:, ttt % 2, jn * SW:(jn + 1) * SW], py[:])
                nc.sync.dma_start(
                    y_v[ttt * P:(ttt + 1) * P, jn * SW:(jn + 1) * SW],
                    ys[:, ttt % 2, jn * SW:(jn + 1) * SW])

            def proj_pieces(slab):
                for ttt in range(4 * slab, 4 * slab + 4):
                    for jn in range(C // SW):
                        def piece(ttt=ttt, jn=jn):
                            py = ps_u.tile([P, SW], F32, tag="ps")
                            proj_mms(ttt, jn, py, 0, ET)
                            proj_store(ttt, jn, py)
                        yield piece

            # ---------------- prologue: DMAs + first fills ----------------
            # all input DMA on the sync hwdge queue (frees the scalar queue
            # for exp); ordered so the first Q/K fill halves start early
            x_dma(0, 0, 2)()
            nc.sync.dma_start(wq_s[:, 0, 0:4, :], wq_v[:, 0, 0:4, :])
            x_dma(0, 2, 4)()
            nc.sync.dma_start(wk_s[:, 0, 0:4, :], wk_v[:, 0, 0:4, :])
            x_dma(0, 4, 6)()
            nc.sync.dma_start(wq_s[:, 0, 4:8, :], wq_v[:, 0, 4:8, :])
            x_dma(0, 6, 8)()
            nc.sync.dma_start(wk_s[:, 0, 4:8, :], wk_v[:, 0, 4:8, :])
            nc.sync.dma_start(wv_s[:, 0:2, :], wv_v[:, 0:2, :])
            nc.sync.dma_start(wq_s[:, 1, :, :], wq_v[:, 1, :, :])
            nc.sync.dma_start(wv_s[:, 2:4, :], wv_v[:, 2:4, :])
            nc.sync.dma_start(wk_s[:, 1, :, :], wk_v[:, 1, :, :])
            nc.sync.dma_start(wv_s[:, 4:6, :], wv_v[:, 4:6, :])
            nc.sync.dma_start(wq_s[:, 2, :, :], wq_v[:, 2, :, :])
            nc.sync.dma_start(wv_s[:, 6:8, :], wv_v[:, 6:8, :])
            nc.sync.dma_start(wk_s[:, 2, :, :], wk_v[:, 2, :, :])
            nc.sync.dma_start(wq_s[:, 3, :, :], wq_v[:, 3, :, :])
            nc.sync.dma_start(wk_s[:, 3, :, :], wk_v[:, 3, :, :])

            def qk_fill_split(w_s, dst):
                xs = xs_map[0]
                pq = ps_u.tile([P, SW], F32, tag="ps")
                for ko in range(4):
                    cmm(pq[:], w_s[:, 0, ko, :], xs[:, ko, :], ko == 0, False)

                def finish():
                    for ko in range(4, KO):
                        cmm(pq[:], w_s[:, 0, ko, :], xs[:, ko, :],
                            False, ko == KO - 1)
                    nc.vector.tensor_copy(dst[:, 0, 0:SW], pq[:])
                return finish

            q0_fin = qk_fill_split(wq_s, QT)
            k0_fin = qk_fill_split(wk_s, KT)
            q0_fin()
            k0_fin()
            v_fill(0, 0)()
            v_fill(0, 1)()

            # ---------------- attention with paced fillers ----------------
            av_q = deque()

            def flush(n_keep):
                while len(av_q) > n_keep:
                    pieces = av_q.popleft()
                    for p in pieces:
                        p()

            class Pacer:
                def __init__(self, items, n_steps, reserve=0):
                    self.q = deque(items)
                    self.rate = max(0, len(items) - reserve) / max(1, n_steps)
                    self.acc = 0.0

                def barrier(self, key):
                    keep = deque()
                    while self.q:
                        piece, dl = self.q.popleft()
                        if dl is not None and dl <= key:
                            piece()
                        else:
                            keep.append((piece, dl))
                    self.q = keep

                def step(self):
                    self.acc += self.rate
                    while self.acc >= 1.0 and self.q:
                        self.q.popleft()[0]()
                        self.acc -= 1.0

                def drain(self, limit=None):
                    n = 0
                    while self.q and (limit is None or n < limit):
                        self.q.popleft()[0]()
                        n += 1

            pending = []
            last_blk = {}

            def block(j, pr, pacer):
                # batch m: scores(2m), scores(2m+1) row-tiled back-to-back,
                # then (via lag) AV slot per sc + one denominator slot, all
                # col-tiled -> exactly 2 mode switches per batch.
                n_sc = 4 * j + 4
                t0 = j * SW
                po = ps_o.tile([P, SW], F32, tag="po")
                dn = ps_o.tile([P, SW], F32, tag="dn")
                last_blk["po"], last_blk["dn"] = po, dn
                for m in range(n_sc // 2):
                    pacer.barrier((j, pr, m))
                    pts = []
                    for sc in (2 * m, 2 * m + 1):
                        dlt = sc * P - t0
                        e0 = max(0, dlt)
                        pw = ps_w.tile([P, 2 * SW], F32, tag="pw")
                        nc.tensor.matmul(
                            pw[:, e0:SW],
                            lhsT=KT[0:D, pr, sc * P:(sc + 1) * P],
                            rhs=QT[0:D, pr, t0 + e0:t0 + SW],
                            start=True, stop=True)
                        nc.tensor.matmul(
                            pw[:, SW:2 * SW - e0],
                            lhsT=KT[D:P, pr, sc * P:(sc + 1) * P],
                            rhs=QT[D:P, pr, t0 + e0:t0 + SW],
                            start=True, stop=True)
                        pt = pt_pool.tile([P, 2 * SW], BF16, tag="pt")
                        nc.scalar.activation(
                            pt[:, e0:2 * SW - e0], pw[:, e0:2 * SW - e0],
                            EXP, scale=SCALE)
                        if dlt >= 0:
                            nc.gpsimd.affine_select(
                                out=pt[:, e0:e0 + P], in_=pt[:, e0:e0 + P],
                                pattern=[[1, P]], compare_op=GE,
                                fill=0.0, base=0, channel_multiplier=-1)
                            nc.gpsimd.affine_select(
                                out=pt[:, SW:SW + P], in_=pt[:, SW:SW + P],
                                pattern=[[1, P]], compare_op=GE,
                                fill=0.0, base=0, channel_multiplier=-1)
                        pts.append((pt, sc, e0))

                    def mk_av(pts=pts, po=po, dn=dn, pr=pr, m=m,
                              n_sc=n_sc):
                        def emit():
                            for pt, sc, e0 in pts:
                                for h in (0, 1):
                                    rhs = (pt[:, e0:SW] if h == 0
                                           else pt[:, SW:2 * SW - e0])
                                    for st in (0, 1):
                                        q = 2 * h + st
                                        nc.tensor.matmul(
                                            po[32 * q:32 * (q + 1), e0:SW],
                                            lhsT=Vsb[:, sc, 2 * pr + h,
                                                     32 * st:32 * (st + 1)],
                                            rhs=rhs,
                                            start=(sc == 0),
                                            stop=(sc == n_sc - 1),
                                            tile_position=(0, 32 * q))
                            # denominator slot: strip q = (slot q>>1,
                            # head q&1), accumulated over batches
                            for s, (pt, sc, e0) in enumerate(pts):
                                for h in (0, 1):
                                    q = 2 * s + h
                                    rhs = (pt[:, e0:SW] if h == 0
                                           else pt[:, SW:2 * SW - e0])
                                    if m == 0 and e0 > 0:
                                        # first write of this strip: clear
                                        # the causal-masked cols [0:e0) so
                                        # the block-end reduce reads zeros
                                        nc.tensor.matmul(
                                            dn[32 * q:32 * (q + 1), 0:e0],
                                            lhsT=ones32[:],
                                            rhs=dm[:, 0:e0],
                                            start=True, stop=False,
                                            tile_position=(0, 32 * q))
                                    nc.tensor.matmul(
                                        dn[32 * q:32 * (q + 1), e0:SW],
                                        lhsT=ones32[:],
                                        rhs=rhs,
                                        start=(m == 0 and e0 == 0),
                                        stop=(m == n_sc // 2 - 1),
                                        tile_position=(0, 32 * q))
                        return emit

                    def mk_norm(po=po, dn=dn, pr=pr, t0=t0):
                        # den strips -> sbuf (bf16), mask reduce-matmul (in
                        # place) into num-aligned layout, then recip + mul
                        dsb = r_pool.tile([P, SW], BF16, tag="db")
                        rcp = r_pool.tile([P, SW], F32, tag="rc")

                        def emit_a():
                            nc.vector.tensor_copy(dsb[:], dn[:])

                        def emit_b():
                            cmm(dn[:], mask32, dsb[:], True, True)
                            nc.vector.reciprocal_approx_fast(
                                out=rcp[:], in_=dn[:])
                            nc.vector.tensor_mul(
                                ot_w(pr, t0, SW), po[:], rcp[:])
                        return emit_a, emit_b

                    entry = []
                    if pending:
                        entry.extend(pending)
                        pending.clear()
                    entry.append(mk_av())
                    last = m == n_sc // 2 - 1
                    if last and not (j == NSLAB - 1 and pr == ET - 1):
                        na, nb = mk_norm()
                        entry.append(na)
                        pending.append(nb)
                    av_q.append(tuple(entry))
                    flush(LOOKAHEAD)
                    pacer.step()

            # phase filler lists: (piece, deadline (j, pr, m) or None)
            Q = {(s, e): qk_fill(s, e, wq_s, QT)
                 for s in range(NSLAB) for e in range(ET)}
            K = {(s, e): qk_fill(s, e, wk_s, KT)
                 for s in range(NSLAB) for e in range(ET)}
            V = {(s, i): v_fill(s, i)
                 for s in range(NSLAB) for i in range(4)}
            phase0 = (
                [(V[0, 2], (0, 1, 0)), (V[0, 3], (0, 1, 0)),
                 (Q[0, 1], (0, 0, 1)), (K[0, 1], (0, 0, 1)),
                 (x_dma(1, 0, 2), None), (x_dma(1, 2, 4), None),
                 (x_dma(1, 4, 6), None), (x_dma(1, 6, 8), None),
                 (Q[0, 2], (0, 1, 1)), (K[0, 2], (0, 1, 1)),
                 (Q[0, 3], (0, 2, 1)), (K[0, 3], (0, 2, 1))]
                + [(K[1, e], None) for e in range(ET)]
                + [(V[1, i], None) for i in range(4)]
                + [(Q[1, 0], None)])
            phase1 = (
                [(Q[1, 1], (1, 0, 1)), (Q[1, 2], (1, 1, 1)),
                 (Q[1, 3], (1, 2, 1)), (x_dma(2, 0, 2), None),
                 (x_dma(2, 2, 4), None), (x_dma(2, 4, 6), None),
                 (x_dma(2, 6, 8), None), (Q[2, 0], None)]
                + [(K[2, e], None) for e in range(ET)]
                + [(V[2, 2], None), (V[2, 3], None)]
                + [(wo_dma(e), None) for e in range(ET)])
            pr01 = list(proj_pieces(0)) + list(proj_pieces(1))
            phase2 = (
                [(x_dma(3, 0, 2), None), (x_dma(3, 2, 4), None),
                 (x_dma(3, 4, 6), None), (x_dma(3, 6, 8), None),
                 (V[2, 0], (2, 0, 4)), (V[2, 1], (2, 0, 4)),
                 (Q[2, 1], (2, 0, 3)), (Q[2, 2], (2, 1, 3)),
                 (Q[2, 3], (2, 2, 3)), (Q[3, 0], None)]
                + [(p, None) for p in pr01[:8]])

            # tail pieces for proj slab 3: a = ko 0..2 accumulation, b = ko 3
            # (waits only its own OT3c chunk) + store
            tails = [(ttt, jn) for ttt in range(12, 16)
                     for jn in range(C // SW)]
            pys = {}

            def tail_a(ttt, jn):
                def piece():
                    py = ps_u.tile([P, SW], F32, tag="ps")
                    proj_mms(ttt, jn, py, 0, ET - 1)
                    pys[(ttt, jn)] = py
                return piece

            def tail_b(ttt, jn):
                py = pys.pop((ttt, jn))
                proj_mms(ttt, jn, py, ET - 1, ET)
                proj_store(ttt, jn, py)

            phase3 = (
                [(Q[3, 1], (3, 0, 2)), (Q[3, 2], (3, 1, 2)),
                 (Q[3, 3], (3, 2, 2)),
                 (K[3, 0], (3, 0, 5)), (K[3, 1], (3, 1, 5)),
                 (K[3, 2], (3, 2, 5)), (K[3, 3], (3, 3, 5)),
                 (V[3, 0], (3, 0, 6)), (V[3, 1], (3, 0, 6)),
                 (V[3, 2], (3, 0, 6)), (V[3, 3], (3, 0, 6))]
                + [(p, None) for p in pr01[8:]]
                + [(p, None) for p in proj_pieces(2)]
                + [(tail_a(12, 0), None), (tail_a(12, 1), None)])
            phases = [phase0, phase1, phase2, phase3]

            for j in range(NSLAB):
                pacer = Pacer(phases[j], 4 * (2 * j + 2),
                              reserve=2 if j == NSLAB - 1 else 0)
                for pr in range(ET):
                    block(j, pr, pacer)
                if j < NSLAB - 1:
                    pacer.drain()
            # the reserved tail_a(12,*) pieces run while the final exp/AV
            # chain drains (they depend only on OTp[0..2] slab-3 writes)
            pacer.drain()
            flush(0)

            # ---- tail: chunked normalize of block (3,3) pipelined with the
            # ko=3 tail matmuls (reduce in place into the den bank)
            po_l, dn_l = last_blk["po"], last_blk["dn"]
            for c in range(4):
                dsb = r_pool.tile([P, P], BF16, tag="db")
                rcp = r_pool.tile([P, P], F32, tag="rc")
                cc = slice(c * P, (c + 1) * P)
                nc.vector.tensor_copy(dsb[:], dn_l[:, cc])
                cmm(dn_l[:, cc], mask32, dsb[:], True, True)
                nc.vector.reciprocal_approx_fast(out=rcp[:], in_=dn_l[:, cc])
                nc.vector.tensor_mul(OT3c[c][:], po_l[:, cc], rcp[:])

            for i in range(len(tails)):
                tail_b(*tails[i])
                if i + 2 < len(tails):
                    tail_a(*tails[i + 2])()
    nc.compile()
    return nc


def _get_nc():
    if "nc" not in _CACHE:
        _CACHE["nc"] = _build()
    return _CACHE["nc"]


_RBKS_ORIG = run_bass_kernel_spmd


def _build_runner(nc, n_cores=8):
    """Cached shard_map executable: run_bass_via_pjrt rebuilds the jit every
    call (seconds of retrace); this builds it once and reuses it."""
    import jax
    import concourse.mybir as mb
    from concourse import bass2jax

    bass2jax.install_neuronx_cc_hook()
    assert nc.dbg_addr is None
    pname = nc.partition_id_tensor.name if nc.partition_id_tensor else None

    in_names, out_names, out_avals, zero_shapes = [], [], [], []
    for alloc in nc.m.functions[0].allocations:
        if not isinstance(alloc, mb.MemoryLocationSet):
            continue
        name = alloc.memorylocations[0].name
        if alloc.kind == "ExternalInput":
            if name != pname:
                in_names.append(name)
        elif alloc.kind == "ExternalOutput":
            out_names.append(name)
            shape = tuple(alloc.tensor_shape)
            dtype = mb.dt.np(alloc.dtype)
            out_avals.append(jax.core.ShapedArray(shape, dtype))
            zero_shapes.append((shape, dtype))
    n_params = len(in_names)
    all_names = list(in_names) + list(out_names)
    if pname is not None:
        all_names.append(pname)
    all_names = tuple(all_names)
    donate = tuple(range(n_params, n_params + len(out_names)))

    def _body(*args):
        operands = list(args)
        if pname is not None:
            operands.append(bass2jax.partition_id_tensor())
        outs = bass2jax._bass_exec_p.bind(
            *operands, out_avals=tuple(out_avals), in_names=all_names,
            out_names=tuple(out_names), lowering_input_output_aliases=(),
            sim_require_finite=True, sim_require_nnan=True, nc=nc)
        return tuple(outs)

    import jax.numpy as jnp
    from jax.sharding import NamedSharding

    devices = jax.devices()[:n_cores]
    mesh = bass2jax.Mesh(np.asarray(devices), ("core",))
    specs = (bass2jax.PartitionSpec("core"),) * (n_params + len(out_names))
    sharded = jax.jit(
        bass2jax.shard_map(_body, mesh=mesh, in_specs=specs,
                           out_specs=specs[:len(out_names)], check_rep=False),
        donate_argnums=donate, keep_unused=True)

    zshard = NamedSharding(mesh, bass2jax.PartitionSpec("core"))
    zeros_maker = jax.jit(
        lambda: tuple(
            jnp.zeros((n_cores * sh[0], *sh[1:]), d) for sh, d in zero_shapes),
        out_shardings=tuple(zshard for _ in zero_shapes))

    def run(in_maps):
        concat_in = [
            np.concatenate([np.asarray(m[k]) for m in in_maps], axis=0)
            for k in in_names]
        out_arrs = sharded(*concat_in, *zeros_maker())
        return [
            {k: np.asarray(out_arrs[i]).reshape(n_cores, *out_avals[i].shape)[c]
             for i, k in enumerate(out_names)}
            for c in range(n_cores)]

    return run


def _run_spmd(nc, in_maps):
    if run_bass_kernel_spmd is not _RBKS_ORIG:
        # externally patched (e.g. tracing harness) — honor it
        res = run_bass_kernel_spmd(nc, in_maps, core_ids=list(range(8)))
        _CACHE["last_result"] = res
        return res.results
    try:
        if "runner" not in _CACHE:
            _CACHE["runner"] = _build_runner(nc)
        return _CACHE["runner"](in_maps)
    except Exception:
        _CACHE.pop("runner", None)
        res = _RBKS_ORIG(nc, in_maps, core_ids=list(range(8)))
        _CACHE["last_result"] = res
        return res.results


def kernel(x, Wq, Wk, Wv, Wp, bp):
    x = np.asarray(x, dtype=np.float32)
    Wq = np.asarray(Wq, dtype=np.float32)
    Wk = np.asarray(Wk, dtype=np.float32)
    Wv = np.asarray(Wv, dtype=np.float32)
    Wp = np.asarray(Wp, dtype=np.float32)
    bp = np.asarray(bp, dtype=np.float32)

    def pack_w(w, g):
        # [H,C,D] head-group g -> [C, E] -> [p, ko, e] partition-major
        m = w[g * NH:(g + 1) * NH].transpose(1, 0, 2).reshape(C, E)
        return np.ascontiguousarray(
            m.reshape(KO, P, E).transpose(1, 0, 2)).astype(BF16NP)

    def pack_w_et(w, g):
        # [H,C,D] head-group g -> [C, E] -> [p, et, ko, d] e-tile-major
        m = w[g * NH:(g + 1) * NH].transpose(1, 0, 2).reshape(C, E)
        return np.ascontiguousarray(
            m.reshape(KO, P, ET, P).transpose(1, 2, 0, 3)).astype(BF16NP)

    nc = _get_nc()
    in_maps = []
    for c in range(8):
        b, g = c // 2, c % 2
        xt = x[b].T  # [C, T]
        xt_r = np.ascontiguousarray(
            xt.reshape(KO, P, NSLAB, SW).transpose(1, 2, 0, 3)).astype(BF16NP)
        wo = Wp[:, g * E:(g + 1) * E].T  # [E, C]
        wo_r = np.ascontiguousarray(
            wo.reshape(ET, P, C).transpose(1, 0, 2)).astype(BF16NP)
        in_maps.append({
            "xt": xt_r,
            "wq": pack_w_et(Wq, g),
            "wk": pack_w_et(Wk, g),
            "wv": pack_w(Wv, g),
            "wo": wo_r,
        })
    results = _run_spmd(nc, in_maps)
    y = np.empty((B, T, C), dtype=np.float32)
    for b in range(B):
        y[b] = (results[2 * b]["y"].astype(np.float32)
                + results[2 * b + 1]["y"].astype(np.float32) + bp)
    return y
